# revision 10
# baseline (speedup 1.0000x reference)
"""Trainium2 Bass kernel for nn_ContrastiveLossV2 (8-core SPMD).

Reference computation:
    z = l2norm(concat([emb_i, emb_j]))          # [8192, 128]
    sim = z @ z.T                               # [8192, 8192]
    loss = mean((sim - class_pairs)**2)

Instead of materializing sim and running an elementwise (sim-cp)^2 pass
(8.4M elements/core on the vector/scalar engines — slower than the DMA
roofline), expand the square:

    sum((sim-cp)^2) = sum(sim^2) - 2*sum(sim*cp) + sum(cp^2)

  * sum(sim^2)  = ||Z^T Z||_F^2   (Gram trick; per-core G_c = Z_c^T Z_c over
                  its 1024 local rows, G = sum_c G_c summed on host in f64)
  * sum(sim*cp) = sum_{d,c} V[d,c] * zT[d,c]  with  V = Z_loc^T @ CP_loc —
                  computed by the TensorEngine directly from *row-major* cp
                  tiles (contraction over the local row dim), so cp never
                  needs a transpose. The dot against zT is a fused
                  scalar_tensor_tensor (mult + accumulate) per PSUM chunk.
  * sum(cp^2)   = Square+accumulate passes, split between the scalar and
                  vector engines.

Row sharding: core c owns reps rows [c*1024, (c+1)*1024) (cores 0-3 from
emb_i, 4-7 from emb_j) and the matching 1024-row slice of class_pairs.
Everything per-core-specific arrives as data (emb_loc / cp_loc inputs), so
one SPMD program serves all 8 cores. class_pairs is cast f32->bf16 in-flight
by the SWDGE DMA; all matmuls run in bf16 with f32 PSUM accumulation.
Per-core partial sums return as small f32 tensors; the host combines them in
f64. The kernel is DMA-bound: 32MB of class_pairs per core at ~358 GB/s.
The small staging inputs (emb_i/emb_j/emb_loc) are explicitly ordered ahead
of the class_pairs stream so the normalization prep is off the critical path.
"""

import numpy as np

import concourse.bacc as bacc
import concourse.mybir as mybir
import concourse.tile as tile
from concourse.bass_utils import run_bass_kernel_spmd
from concourse.tile_rust import add_dep_helper

f32 = mybir.dt.float32
bf16 = mybir.dt.bfloat16
AF = mybir.ActivationFunctionType
OP = mybir.AluOpType

N_CORES = 8
N, D = 4096, 128
TWO_N = 2 * N                     # 8192
R_LOC = TWO_N // N_CORES          # 1024 rows per core
M_BLK = R_LOC // 128              # 8 local 128-row blocks
NCH = 512                         # matmul free-dim chunk (one PSUM bank)
N4 = 2048                         # cp DMA tile width (1MB f32 reads)
N_GRP = TWO_N // N4               # 4 column groups
K_PER_GRP = N4 // NCH             # 4 chunks per group
EPS = 1e-12
# how many of the 8 per-group cp^2 squares run on the vector engine
# (the rest run on the scalar engine): per-group DVE/ACT load balance knob
CP2_ON_DVE = 1

_cached = {}


def _build_module():
    nc = bacc.Bacc("TRN2", target_bir_lowering=False, debug=False,
                   num_devices=N_CORES)

    emb_i = nc.dram_tensor("emb_i", [N, D], f32, kind="ExternalInput")
    emb_j = nc.dram_tensor("emb_j", [N, D], f32, kind="ExternalInput")
    emb_loc = nc.dram_tensor("emb_loc", [R_LOC, D], f32, kind="ExternalInput")
    cp_loc = nc.dram_tensor("cp_loc", [R_LOC, TWO_N], f32, kind="ExternalInput")
    ident = nc.dram_tensor("ident", [128, 128], bf16, kind="ExternalInput")

    out_g = nc.dram_tensor("out_g", [128, 128], f32, kind="ExternalOutput")
    out_cp = nc.dram_tensor("out_cp", [128, M_BLK * N_GRP], f32,
                            kind="ExternalOutput")
    out_x = nc.dram_tensor("out_x", [128, N_GRP * K_PER_GRP], f32,
                           kind="ExternalOutput")

    with tile.TileContext(nc) as tc:
        with (
            tc.tile_pool(name="const", bufs=1) as const_pool,
            tc.tile_pool(name="persist", bufs=1) as persist,
            tc.tile_pool(name="stag", bufs=2) as stag_pool,
            tc.tile_pool(name="sq", bufs=2) as sq_pool,
            tc.tile_pool(name="norm", bufs=4) as norm_pool,
            tc.tile_pool(name="zc", bufs=2) as zc_pool,
            tc.tile_pool(name="cpt", bufs=16) as cp_pool,
            tc.tile_pool(name="tmp", bufs=4) as tmp_pool,
            tc.tile_pool(name="sqj", bufs=2) as sqj_pool,
            tc.tile_pool(name="psv", bufs=5, space="PSUM") as psv_pool,
            tc.tile_pool(name="pst", bufs=2, space="PSUM") as pst_pool,
            tc.tile_pool(name="psg", bufs=1, space="PSUM") as psg_pool,
        ):
            # zT[d, r] = normalized reps row r, transposed. bf16.
            zT = persist.tile([128, TWO_N], bf16)
            # local row-major z tiles (natural 128-row blocks), bf16.
            z_loc = persist.tile([128, M_BLK, 128], bf16)
            # accumulators for the partial sums
            acc_cp = persist.tile([128, M_BLK * N_GRP], f32)
            acc_x = persist.tile([128, N_GRP * K_PER_GRP], f32)

            # All input loads go through the SWDGE (gpsimd) queue in strict
            # order: local staging first (unblocks the V matmuls), then the
            # full embs (unblock zT), then the 32MB class_pairs stream.
            stage_dmas = []
            stag_l = stag_pool.tile([128, M_BLK, 128], f32, tag="stag_loc",
                                    name="stag_loc")
            stage_dmas.append(nc.gpsimd.dma_start(
                out=stag_l[:],
                in_=emb_loc[:].rearrange("(n p) d -> p n d", p=128)))
            stags = []
            for emb in (emb_i, emb_j):
                stag = stag_pool.tile([128, 32, 128], f32, tag="stag",
                                      name="stag")
                # (n p) d -> p n d: chunk n holds row n*128+p (natural
                # 128-row blocks, matching cp column order).
                stage_dmas.append(nc.gpsimd.dma_start(
                    out=stag[:],
                    in_=emb[:].rearrange("(n p) d -> p n d", p=128)))
                stags.append(stag)
            ident_sb = const_pool.tile([128, 128], bf16)
            stage_dmas.append(nc.gpsimd.dma_start(out=ident_sb[:], in_=ident[:]))

            def normalize(stag, n_chunks, z_out):
                """stag: [128, n_chunks, 128] f32 staging; chunk n is the
                natural 128-row block n (row n*128+p on partition p). Writes
                the row-normalized bf16 result into z_out [128, n_chunks, 128]
                with a single broadcast multiply."""
                sq = sq_pool.tile([128, n_chunks, 128], f32, tag="sq",
                                  name="sq")
                nc.vector.tensor_tensor(sq[:], stag[:], stag[:], op=OP.mult)
                nsq = norm_pool.tile([128, n_chunks], f32, tag="nsq",
                                     name="nsq")
                nc.vector.tensor_reduce(nsq[:], sq[:], axis=mybir.AxisListType.X,
                                        op=OP.add)
                nrm = norm_pool.tile([128, n_chunks], f32, tag="nrm",
                                     name="nrm")
                nc.scalar.activation(nrm[:], nsq[:], AF.Sqrt)
                nc.vector.tensor_scalar_max(nrm[:], nrm[:], EPS)
                rec = norm_pool.tile([128, n_chunks], f32, tag="rec",
                                     name="rec")
                nc.vector.reciprocal(rec[:], nrm[:])
                rec_b = rec[:].rearrange("q n -> q n ()") \
                    .broadcast_to([128, n_chunks, 128])
                nc.vector.tensor_tensor(z_out, stag[:], rec_b, op=OP.mult)

            # ---- phase A: local row blocks (natural order) ----
            normalize(stag_l, M_BLK, z_loc[:])

            # ---- phase B: build zT from emb_i / emb_j ----
            for ei, stag in enumerate(stags):
                base = ei * N
                zbig = zc_pool.tile([128, 32, 128], bf16, tag="zbig",
                                    name="zbig")
                normalize(stag, 32, zbig[:])
                for g in range(8):
                    ps4 = pst_pool.tile([128, 4, 128], bf16, tag="ps4",
                                        name="ps4")
                    for dlt in range(4):
                        nc.tensor.transpose(ps4[:, dlt, :],
                                            zbig[:, 4 * g + dlt, :], ident_sb[:])
                    # chunks 4g..4g+3 transpose to zT columns
                    # [base+512g, base+512g+512), contiguous.
                    nc.vector.tensor_copy(
                        zT[:, base + 512 * g: base + 512 * (g + 1)]
                        .rearrange("q (n p) -> q n p", n=4),
                        ps4[:])

            # ---- phase C: G = Z_loc^T @ Z_loc (local Gram, 128x128) ----
            g_ps = psg_pool.tile([128, 128], f32)
            for m in range(M_BLK):
                nc.tensor.matmul(g_ps[:], lhsT=z_loc[:, m, :], rhs=z_loc[:, m, :],
                                 start=(m == 0), stop=(m == M_BLK - 1))
            g_sb = tmp_pool.tile([128, 128], f32, tag="gsb")
            nc.scalar.copy(g_sb[:], g_ps[:])
            nc.sync.dma_start(out=out_g[:], in_=g_sb[:])

            # ---- phase D: stream class_pairs ----
            first_cp_dma = None
            for n4 in range(N_GRP):
                cpts = []
                for m in range(M_BLK):
                    cpt = cp_pool.tile([128, N4], bf16, tag="cpt", name="cpt")
                    # SWDGE cast f32 -> bf16 in flight
                    d = nc.gpsimd.dma_start(
                        out=cpt[:],
                        in_=cp_loc[m * 128:(m + 1) * 128,
                                   n4 * N4:(n4 + 1) * N4])
                    if first_cp_dma is None:
                        first_cp_dma = d
                        # keep the small staging inputs ahead of the 32MB
                        # class_pairs stream so prep finishes early
                        for sd in stage_dmas:
                            add_dep_helper(sd.ins, d.ins, True,
                                           "stage inputs before cp stream")
                    cpts.append(cpt)
                for k in range(K_PER_GRP):
                    ps = psv_pool.tile([128, NCH], f32, tag="psv", name="psv")
                    for m in range(M_BLK):
                        nc.tensor.matmul(ps[:], lhsT=z_loc[:, m, :],
                                         rhs=cpts[m][:, k * NCH:(k + 1) * NCH],
                                         start=(m == 0), stop=(m == M_BLK - 1))
                    col0 = n4 * N4 + k * NCH
                    xj = tmp_pool.tile([128, NCH], bf16, tag="xj", name="xj")
                    # acc_x[:, t] = sum_c ps[:, c] * zT[:, col0 + c]
                    nc.vector.scalar_tensor_tensor(
                        out=xj[:], in0=ps[:], scalar=1.0,
                        in1=zT[:, col0:col0 + NCH],
                        op0=OP.mult, op1=OP.mult,
                        accum_out=acc_x[:, n4 * K_PER_GRP + k:
                                        n4 * K_PER_GRP + k + 1])
                for m in range(M_BLK):
                    idx = n4 * M_BLK + m
                    if m < CP2_ON_DVE:
                        sj = sqj_pool.tile([128, N4], bf16, tag="sjv",
                                           name="sjv")
                        nc.vector.scalar_tensor_tensor(
                            out=sj[:], in0=cpts[m][:], scalar=1.0,
                            in1=cpts[m][:], op0=OP.mult, op1=OP.mult,
                            accum_out=acc_cp[:, idx:idx + 1])
                    else:
                        sj = sqj_pool.tile([128, N4], bf16, tag="sja",
                                           name="sja")
                        nc.scalar.activation(sj[:], cpts[m][:], AF.Square,
                                             accum_out=acc_cp[:, idx:idx + 1])

            nc.sync.dma_start(out=out_cp[:], in_=acc_cp[:])
            nc.sync.dma_start(out=out_x[:], in_=acc_x[:])

    nc.compile()
    return nc


def _get_module():
    if "nc" not in _cached:
        _cached["nc"] = _build_module()
    return _cached["nc"]


def kernel(emb_i, emb_j, class_pairs, _return_raw=False, _trace=False):
    import ml_dtypes

    emb_i = np.ascontiguousarray(emb_i, dtype=np.float32)
    emb_j = np.ascontiguousarray(emb_j, dtype=np.float32)
    class_pairs = np.ascontiguousarray(class_pairs, dtype=np.float32)
    ident = np.eye(128, dtype=ml_dtypes.bfloat16)

    nc = _get_module()
    in_maps = []
    for c in range(N_CORES):
        r0 = c * R_LOC
        if r0 < N:
            emb_loc = emb_i[r0:r0 + R_LOC]
        else:
            emb_loc = emb_j[r0 - N:r0 - N + R_LOC]
        in_maps.append({
            "emb_i": emb_i,
            "emb_j": emb_j,
            "emb_loc": np.ascontiguousarray(emb_loc),
            "cp_loc": np.ascontiguousarray(class_pairs[r0:r0 + R_LOC]),
            "ident": ident,
        })

    res = run_bass_kernel_spmd(nc, in_maps, list(range(N_CORES)), trace=_trace)

    G = np.zeros((128, 128), dtype=np.float64)
    sum_cp2 = 0.0
    cross = 0.0
    for c in range(N_CORES):
        G += res.results[c]["out_g"].astype(np.float64)
        sum_cp2 += res.results[c]["out_cp"].astype(np.float64).sum()
        cross += res.results[c]["out_x"].astype(np.float64).sum()
    sum_sim2 = float((G * G).sum())
    loss = (sum_sim2 - 2.0 * cross + sum_cp2) / float(TWO_N * TWO_N)
    out = np.asarray(loss, dtype=np.float32)
    if _return_raw:
        return out, res
    return out


# revision 11
# speedup vs baseline: 1.0248x; 1.0248x over previous
"""Trainium2 Bass kernel for nn_ContrastiveLossV2 (8-core SPMD).

Reference computation:
    z = l2norm(concat([emb_i, emb_j]))          # [8192, 128]
    sim = z @ z.T                               # [8192, 8192]
    loss = mean((sim - class_pairs)**2)

Instead of materializing sim and running an elementwise (sim-cp)^2 pass
(8.4M elements/core on the vector/scalar engines — slower than the DMA
roofline), expand the square:

    sum((sim-cp)^2) = sum(sim^2) - 2*sum(sim*cp) + sum(cp^2)

  * sum(sim^2)  = ||Z^T Z||_F^2   (Gram trick; per-core G_c = Z_c^T Z_c over
                  its 1024 local rows, G = sum_c G_c summed on host in f64)
  * sum(sim*cp) = sum_{d,c} V[d,c] * zT[d,c]  with  V = Z_loc^T @ CP_loc —
                  computed by the TensorEngine directly from *row-major* cp
                  tiles (contraction over the local row dim), so cp never
                  needs a transpose. The dot against zT is a fused
                  scalar_tensor_tensor (mult + accumulate) per PSUM chunk.
  * sum(cp^2)   = Square+accumulate passes, split between the scalar and
                  vector engines.

Row sharding: core c owns reps rows [c*1024, (c+1)*1024) (cores 0-3 from
emb_i, 4-7 from emb_j) and the matching 1024-row slice of class_pairs.
Everything per-core-specific arrives as data (emb_loc / cp_loc inputs), so
one SPMD program serves all 8 cores. class_pairs is cast f32->bf16 in-flight
by the SWDGE DMA; all matmuls run in bf16 with f32 PSUM accumulation.
Per-core partial sums return as small f32 tensors; the host combines them in
f64. The kernel is DMA-bound: 32MB of class_pairs per core at ~358 GB/s.
The small staging inputs (emb_i/emb_j/emb_loc) are explicitly ordered ahead
of the class_pairs stream so the normalization prep is off the critical path.
"""

import numpy as np

import concourse.bacc as bacc
import concourse.mybir as mybir
import concourse.tile as tile
from concourse.bass_utils import run_bass_kernel_spmd
from concourse.tile_rust import add_dep_helper

f32 = mybir.dt.float32
bf16 = mybir.dt.bfloat16
AF = mybir.ActivationFunctionType
OP = mybir.AluOpType

N_CORES = 8
N, D = 4096, 128
TWO_N = 2 * N                     # 8192
R_LOC = TWO_N // N_CORES          # 1024 rows per core
M_BLK = R_LOC // 128              # 8 local 128-row blocks
NCH = 512                         # matmul free-dim chunk (one PSUM bank)
N4 = 2048                         # cp DMA tile width (1MB f32 reads)
N_GRP = TWO_N // N4               # 4 column groups
K_PER_GRP = N4 // NCH             # 4 chunks per group
EPS = 1e-12
# how many of the 8 per-group cp^2 squares run on the vector engine
# (the rest run on the scalar engine): per-group DVE/ACT load balance knob
CP2_ON_DVE = 2

_cached = {}


def _build_module():
    nc = bacc.Bacc("TRN2", target_bir_lowering=False, debug=False,
                   num_devices=N_CORES)

    emb_i = nc.dram_tensor("emb_i", [N, D], f32, kind="ExternalInput")
    emb_j = nc.dram_tensor("emb_j", [N, D], f32, kind="ExternalInput")
    emb_loc = nc.dram_tensor("emb_loc", [R_LOC, D], f32, kind="ExternalInput")
    cp_loc = nc.dram_tensor("cp_loc", [R_LOC, TWO_N], f32, kind="ExternalInput")
    ident = nc.dram_tensor("ident", [128, 128], bf16, kind="ExternalInput")

    out_g = nc.dram_tensor("out_g", [128, 128], f32, kind="ExternalOutput")
    out_cp = nc.dram_tensor("out_cp", [128, M_BLK * N_GRP], f32,
                            kind="ExternalOutput")
    out_x = nc.dram_tensor("out_x", [128, N_GRP * K_PER_GRP], f32,
                           kind="ExternalOutput")

    with tile.TileContext(nc) as tc:
        with (
            tc.tile_pool(name="const", bufs=1) as const_pool,
            tc.tile_pool(name="persist", bufs=1) as persist,
            tc.tile_pool(name="stag", bufs=2) as stag_pool,
            tc.tile_pool(name="sq", bufs=2) as sq_pool,
            tc.tile_pool(name="norm", bufs=4) as norm_pool,
            tc.tile_pool(name="zc", bufs=2) as zc_pool,
            tc.tile_pool(name="cpt", bufs=16) as cp_pool,
            tc.tile_pool(name="tmp", bufs=4) as tmp_pool,
            tc.tile_pool(name="sqj", bufs=2) as sqj_pool,
            tc.tile_pool(name="psv", bufs=5, space="PSUM") as psv_pool,
            tc.tile_pool(name="pst", bufs=2, space="PSUM") as pst_pool,
            tc.tile_pool(name="psg", bufs=1, space="PSUM") as psg_pool,
        ):
            # zT[d, r] = normalized reps row r, transposed. bf16.
            zT = persist.tile([128, TWO_N], bf16)
            # local row-major z tiles (natural 128-row blocks), bf16.
            z_loc = persist.tile([128, M_BLK, 128], bf16)
            # accumulators for the partial sums
            acc_cp = persist.tile([128, M_BLK * N_GRP], f32)
            acc_x = persist.tile([128, N_GRP * K_PER_GRP], f32)

            # All input loads go through the SWDGE (gpsimd) queue in strict
            # order: local staging first (unblocks the V matmuls), then the
            # full embs (unblock zT), then the 32MB class_pairs stream.
            stage_dmas = []
            stag_l = stag_pool.tile([128, M_BLK, 128], f32, tag="stag_loc",
                                    name="stag_loc")
            stage_dmas.append(nc.gpsimd.dma_start(
                out=stag_l[:],
                in_=emb_loc[:].rearrange("(n p) d -> p n d", p=128)))
            stags = []
            for emb in (emb_i, emb_j):
                stag = stag_pool.tile([128, 32, 128], f32, tag="stag",
                                      name="stag")
                # (n p) d -> p n d: chunk n holds row n*128+p (natural
                # 128-row blocks, matching cp column order).
                stage_dmas.append(nc.gpsimd.dma_start(
                    out=stag[:],
                    in_=emb[:].rearrange("(n p) d -> p n d", p=128)))
                stags.append(stag)
            ident_sb = const_pool.tile([128, 128], bf16)
            stage_dmas.append(nc.gpsimd.dma_start(out=ident_sb[:], in_=ident[:]))

            def normalize(stag, n_chunks, z_out):
                """stag: [128, n_chunks, 128] f32 staging; chunk n is the
                natural 128-row block n (row n*128+p on partition p). Writes
                the row-normalized bf16 result into z_out [128, n_chunks, 128]
                with a single broadcast multiply."""
                sq = sq_pool.tile([128, n_chunks, 128], f32, tag="sq",
                                  name="sq")
                nc.vector.tensor_tensor(sq[:], stag[:], stag[:], op=OP.mult)
                nsq = norm_pool.tile([128, n_chunks], f32, tag="nsq",
                                     name="nsq")
                nc.vector.tensor_reduce(nsq[:], sq[:], axis=mybir.AxisListType.X,
                                        op=OP.add)
                nrm = norm_pool.tile([128, n_chunks], f32, tag="nrm",
                                     name="nrm")
                nc.scalar.activation(nrm[:], nsq[:], AF.Sqrt)
                nc.vector.tensor_scalar_max(nrm[:], nrm[:], EPS)
                rec = norm_pool.tile([128, n_chunks], f32, tag="rec",
                                     name="rec")
                nc.vector.reciprocal(rec[:], nrm[:])
                rec_b = rec[:].rearrange("q n -> q n ()") \
                    .broadcast_to([128, n_chunks, 128])
                nc.vector.tensor_tensor(z_out, stag[:], rec_b, op=OP.mult)

            # ---- phase A: local row blocks (natural order) ----
            normalize(stag_l, M_BLK, z_loc[:])

            # ---- phase B: build zT from emb_i / emb_j ----
            for ei, stag in enumerate(stags):
                base = ei * N
                zbig = zc_pool.tile([128, 32, 128], bf16, tag="zbig",
                                    name="zbig")
                normalize(stag, 32, zbig[:])
                for g in range(8):
                    ps4 = pst_pool.tile([128, 4, 128], bf16, tag="ps4",
                                        name="ps4")
                    for dlt in range(4):
                        nc.tensor.transpose(ps4[:, dlt, :],
                                            zbig[:, 4 * g + dlt, :], ident_sb[:])
                    # chunks 4g..4g+3 transpose to zT columns
                    # [base+512g, base+512g+512), contiguous.
                    nc.vector.tensor_copy(
                        zT[:, base + 512 * g: base + 512 * (g + 1)]
                        .rearrange("q (n p) -> q n p", n=4),
                        ps4[:])

            # ---- phase C: G = Z_loc^T @ Z_loc (local Gram, 128x128) ----
            g_ps = psg_pool.tile([128, 128], f32)
            for m in range(M_BLK):
                nc.tensor.matmul(g_ps[:], lhsT=z_loc[:, m, :], rhs=z_loc[:, m, :],
                                 start=(m == 0), stop=(m == M_BLK - 1))
            g_sb = tmp_pool.tile([128, 128], f32, tag="gsb")
            nc.scalar.copy(g_sb[:], g_ps[:])
            nc.sync.dma_start(out=out_g[:], in_=g_sb[:])

            # ---- phase D: stream class_pairs ----
            first_cp_dma = None
            for n4 in range(N_GRP):
                cpts = []
                for m in range(M_BLK):
                    cpt = cp_pool.tile([128, N4], bf16, tag="cpt", name="cpt")
                    # SWDGE cast f32 -> bf16 in flight
                    d = nc.gpsimd.dma_start(
                        out=cpt[:],
                        in_=cp_loc[m * 128:(m + 1) * 128,
                                   n4 * N4:(n4 + 1) * N4])
                    if first_cp_dma is None:
                        first_cp_dma = d
                        # keep the small staging inputs ahead of the 32MB
                        # class_pairs stream so prep finishes early
                        for sd in stage_dmas:
                            add_dep_helper(sd.ins, d.ins, True,
                                           "stage inputs before cp stream")
                    cpts.append(cpt)
                for k in range(K_PER_GRP):
                    ps = psv_pool.tile([128, NCH], f32, tag="psv", name="psv")
                    for m in range(M_BLK):
                        nc.tensor.matmul(ps[:], lhsT=z_loc[:, m, :],
                                         rhs=cpts[m][:, k * NCH:(k + 1) * NCH],
                                         start=(m == 0), stop=(m == M_BLK - 1))
                    col0 = n4 * N4 + k * NCH
                    xj = tmp_pool.tile([128, NCH], bf16, tag="xj", name="xj")
                    # acc_x[:, t] = sum_c ps[:, c] * zT[:, col0 + c]
                    nc.vector.scalar_tensor_tensor(
                        out=xj[:], in0=ps[:], scalar=1.0,
                        in1=zT[:, col0:col0 + NCH],
                        op0=OP.mult, op1=OP.mult,
                        accum_out=acc_x[:, n4 * K_PER_GRP + k:
                                        n4 * K_PER_GRP + k + 1])
                for m in range(M_BLK):
                    idx = n4 * M_BLK + m
                    if m < CP2_ON_DVE:
                        sj = sqj_pool.tile([128, N4], bf16, tag="sjv",
                                           name="sjv")
                        nc.vector.scalar_tensor_tensor(
                            out=sj[:], in0=cpts[m][:], scalar=1.0,
                            in1=cpts[m][:], op0=OP.mult, op1=OP.mult,
                            accum_out=acc_cp[:, idx:idx + 1])
                    else:
                        sj = sqj_pool.tile([128, N4], bf16, tag="sja",
                                           name="sja")
                        nc.scalar.activation(sj[:], cpts[m][:], AF.Square,
                                             accum_out=acc_cp[:, idx:idx + 1])

            nc.sync.dma_start(out=out_cp[:], in_=acc_cp[:])
            nc.sync.dma_start(out=out_x[:], in_=acc_x[:])

    nc.compile()
    return nc


def _get_module():
    if "nc" not in _cached:
        _cached["nc"] = _build_module()
    return _cached["nc"]


def kernel(emb_i, emb_j, class_pairs, _return_raw=False, _trace=False):
    import ml_dtypes

    emb_i = np.ascontiguousarray(emb_i, dtype=np.float32)
    emb_j = np.ascontiguousarray(emb_j, dtype=np.float32)
    class_pairs = np.ascontiguousarray(class_pairs, dtype=np.float32)
    ident = np.eye(128, dtype=ml_dtypes.bfloat16)

    nc = _get_module()
    in_maps = []
    for c in range(N_CORES):
        r0 = c * R_LOC
        if r0 < N:
            emb_loc = emb_i[r0:r0 + R_LOC]
        else:
            emb_loc = emb_j[r0 - N:r0 - N + R_LOC]
        in_maps.append({
            "emb_i": emb_i,
            "emb_j": emb_j,
            "emb_loc": np.ascontiguousarray(emb_loc),
            "cp_loc": np.ascontiguousarray(class_pairs[r0:r0 + R_LOC]),
            "ident": ident,
        })

    res = run_bass_kernel_spmd(nc, in_maps, list(range(N_CORES)), trace=_trace)

    G = np.zeros((128, 128), dtype=np.float64)
    sum_cp2 = 0.0
    cross = 0.0
    for c in range(N_CORES):
        G += res.results[c]["out_g"].astype(np.float64)
        sum_cp2 += res.results[c]["out_cp"].astype(np.float64).sum()
        cross += res.results[c]["out_x"].astype(np.float64).sum()
    sum_sim2 = float((G * G).sum())
    loss = (sum_sim2 - 2.0 * cross + sum_cp2) / float(TWO_N * TWO_N)
    out = np.asarray(loss, dtype=np.float32)
    if _return_raw:
        return out, res
    return out


# revision 20
# speedup vs baseline: 1.2029x; 1.1737x over previous
"""Trainium2 Bass kernel for nn_ContrastiveLossV2 (8-core SPMD).

Reference computation:
    z = l2norm(concat([emb_i, emb_j]))          # [8192, 128]
    sim = z @ z.T                               # [8192, 8192]
    loss = mean((sim - class_pairs)**2)

Instead of materializing sim and running an elementwise (sim-cp)^2 pass
(8.4M elements/core on the vector/scalar engines — slower than the DMA
roofline), expand the square:

    sum((sim-cp)^2) = sum(sim^2) - 2*sum(sim*cp) + sum(cp^2)

  * sum(sim^2)  = ||Z^T Z||_F^2   (Gram trick; per-core G_c = Z_c^T Z_c over
                  its 1024 local rows, G = sum_c G_c summed on host in f64)
  * sum(sim*cp) = sum_{d,c} V[d,c] * zT[d,c]  with  V = Z_loc^T @ CP_loc —
                  computed by the TensorEngine directly from *row-major* cp
                  tiles (contraction over the local row dim), so cp never
                  needs a transpose. The dot against zT is a fused
                  scalar_tensor_tensor (mult + accumulate) per PSUM chunk.
  * sum(cp^2)   = Square+accumulate passes, split between the scalar and
                  vector engines.

Row sharding: core c owns reps rows [c*1024, (c+1)*1024) (cores 0-3 from
emb_i, 4-7 from emb_j) and the matching 1024-row slice of class_pairs.
Everything per-core-specific arrives as data (emb_loc / cp_loc inputs), so
one SPMD program serves all 8 cores. class_pairs is cast f32->bf16 in-flight
by the SWDGE DMA; all matmuls run in bf16 with f32 PSUM accumulation. The
small replicated emb inputs are uploaded pre-staged (bf16, partition-major
device layout) so their DMA is a contiguous read. Per-core partial sums
return as small f32 tensors; the host combines them in f64. The kernel is
DMA-bound: 32MB of class_pairs per core at ~358 GB/s. The staging inputs
are explicitly ordered ahead of the class_pairs stream so the normalization
prep is off the critical path.
"""

import numpy as np

import concourse.bacc as bacc
import concourse.mybir as mybir
import concourse.tile as tile
from concourse.bass_utils import run_bass_kernel_spmd
from concourse.tile_rust import add_dep_helper

f32 = mybir.dt.float32
bf16 = mybir.dt.bfloat16
AF = mybir.ActivationFunctionType
OP = mybir.AluOpType

N_CORES = 8
N, D = 4096, 128
TWO_N = 2 * N                     # 8192
R_LOC = TWO_N // N_CORES          # 1024 rows per core
M_BLK = R_LOC // 128              # 8 local 128-row blocks
NCH = 512                         # matmul free-dim chunk (one PSUM bank)
N4 = 2048                         # cp DMA tile width (1MB f32 reads)
N_GRP = TWO_N // N4               # 4 column groups
K_PER_GRP = N4 // NCH             # 4 chunks per group
EPS = 1e-12
# how many of the 8 per-group cp^2 squares run on the vector engine
# (the rest run on the scalar engine): per-group DVE/ACT load balance knob
CP2_ON_DVE = 2

_cached = {}


def _build_module():
    nc = bacc.Bacc("TRN2", target_bir_lowering=False, debug=False,
                   num_devices=N_CORES)

    # The (small, replicated) embedding inputs are uploaded pre-staged by the
    # host: bf16, already arranged as [partition, block, d] with block n
    # holding row n*128+p on partition p — so the device DMA is a fully
    # contiguous per-partition read instead of a 512B/row gather.
    emb_i = nc.dram_tensor("emb_i", [128, N // 128, D], bf16,
                           kind="ExternalInput")
    emb_j = nc.dram_tensor("emb_j", [128, N // 128, D], bf16,
                           kind="ExternalInput")
    emb_loc = nc.dram_tensor("emb_loc", [128, M_BLK, D], bf16,
                             kind="ExternalInput")
    cp_loc = nc.dram_tensor("cp_loc", [R_LOC, TWO_N], f32, kind="ExternalInput")
    ident = nc.dram_tensor("ident", [128, 128], bf16, kind="ExternalInput")

    out_g = nc.dram_tensor("out_g", [128, 128], f32, kind="ExternalOutput")
    out_cp = nc.dram_tensor("out_cp", [128, M_BLK * N_GRP], f32,
                            kind="ExternalOutput")
    out_x = nc.dram_tensor("out_x", [128, N_GRP * K_PER_GRP], f32,
                           kind="ExternalOutput")

    with tile.TileContext(nc) as tc:
        with (
            tc.tile_pool(name="const", bufs=1) as const_pool,
            tc.tile_pool(name="persist", bufs=1) as persist,
            tc.tile_pool(name="stag", bufs=2) as stag_pool,
            tc.tile_pool(name="sq", bufs=1) as sq_pool,
            tc.tile_pool(name="norm", bufs=4) as norm_pool,
            tc.tile_pool(name="zc", bufs=2) as zc_pool,
            tc.tile_pool(name="cpt", bufs=22) as cp_pool,
            tc.tile_pool(name="tmp", bufs=4) as tmp_pool,
            tc.tile_pool(name="sqj", bufs=2) as sqj_pool,
            tc.tile_pool(name="psv", bufs=5, space="PSUM") as psv_pool,
            tc.tile_pool(name="pst", bufs=2, space="PSUM") as pst_pool,
            tc.tile_pool(name="psg", bufs=1, space="PSUM") as psg_pool,
        ):
            # zT[d, r] = normalized reps row r, transposed. bf16.
            zT = persist.tile([128, TWO_N], bf16)
            # local row-major z tiles (natural 128-row blocks), bf16.
            z_loc = persist.tile([128, M_BLK, 128], bf16)
            # accumulators for the partial sums
            acc_cp = persist.tile([128, M_BLK * N_GRP], f32)
            acc_x = persist.tile([128, N_GRP * K_PER_GRP], f32)

            # All input loads go through the SWDGE (gpsimd) queue in strict
            # order: local staging first (unblocks the V matmuls), then the
            # full embs (unblock zT), then the 32MB class_pairs stream.
            stage_dmas = []
            stag_l = stag_pool.tile([128, M_BLK, 128], bf16, tag="stag_loc",
                                    name="stag_loc")
            stage_dmas.append(nc.gpsimd.dma_start(out=stag_l[:], in_=emb_loc[:]))
            stags = []
            for emb in (emb_i, emb_j):
                stag = stag_pool.tile([128, 32, 128], bf16, tag="stag",
                                      name="stag")
                stage_dmas.append(nc.gpsimd.dma_start(out=stag[:], in_=emb[:]))
                stags.append(stag)
            ident_sb = const_pool.tile([128, 128], bf16)
            stage_dmas.append(nc.gpsimd.dma_start(out=ident_sb[:], in_=ident[:]))

            def normalize(stag, n_chunks, z_out):
                """stag: [128, n_chunks, 128] bf16 staging; chunk n is the
                natural 128-row block n (row n*128+p on partition p). Writes
                the row-normalized bf16 result into z_out [128, n_chunks, 128]
                with a single broadcast multiply (f32 internal math)."""
                sq = sq_pool.tile([128, n_chunks, 128], f32, tag="sq",
                                  name="sq")
                nc.vector.tensor_tensor(sq[:], stag[:], stag[:], op=OP.mult)
                nsq = norm_pool.tile([128, n_chunks], f32, tag="nsq",
                                     name="nsq")
                nc.vector.tensor_reduce(nsq[:], sq[:], axis=mybir.AxisListType.X,
                                        op=OP.add)
                nrm = norm_pool.tile([128, n_chunks], f32, tag="nrm",
                                     name="nrm")
                nc.scalar.activation(nrm[:], nsq[:], AF.Sqrt)
                nc.vector.tensor_scalar_max(nrm[:], nrm[:], EPS)
                rec = norm_pool.tile([128, n_chunks], f32, tag="rec",
                                     name="rec")
                nc.vector.reciprocal(rec[:], nrm[:])
                rec_b = rec[:].rearrange("q n -> q n ()") \
                    .broadcast_to([128, n_chunks, 128])
                nc.vector.tensor_tensor(z_out, stag[:], rec_b, op=OP.mult)

            # ---- phase A: local row blocks (natural order) ----
            normalize(stag_l, M_BLK, z_loc[:])

            # ---- phase B: build zT from emb_i / emb_j ----
            for ei, stag in enumerate(stags):
                base = ei * N
                zbig = zc_pool.tile([128, 32, 128], bf16, tag="zbig",
                                    name="zbig")
                normalize(stag, 32, zbig[:])
                for g in range(8):
                    ps4 = pst_pool.tile([128, 4, 128], bf16, tag="ps4",
                                        name="ps4")
                    for dlt in range(4):
                        nc.tensor.transpose(ps4[:, dlt, :],
                                            zbig[:, 4 * g + dlt, :], ident_sb[:])
                    # chunks 4g..4g+3 transpose to zT columns
                    # [base+512g, base+512g+512), contiguous.
                    nc.vector.tensor_copy(
                        zT[:, base + 512 * g: base + 512 * (g + 1)]
                        .rearrange("q (n p) -> q n p", n=4),
                        ps4[:])

            # ---- phase C: G = Z_loc^T @ Z_loc (local Gram, 128x128) ----
            g_ps = psg_pool.tile([128, 128], f32)
            for m in range(M_BLK):
                nc.tensor.matmul(g_ps[:], lhsT=z_loc[:, m, :], rhs=z_loc[:, m, :],
                                 start=(m == 0), stop=(m == M_BLK - 1))
            g_sb = tmp_pool.tile([128, 128], f32, tag="gsb")
            nc.scalar.copy(g_sb[:], g_ps[:])
            nc.sync.dma_start(out=out_g[:], in_=g_sb[:])

            # ---- phase D: stream class_pairs ----
            first_cp_dma = None
            for n4 in range(N_GRP):
                cpts = []
                for m in range(M_BLK):
                    cpt = cp_pool.tile([128, N4], bf16, tag="cpt", name="cpt")
                    # SWDGE cast f32 -> bf16 in flight
                    d = nc.gpsimd.dma_start(
                        out=cpt[:],
                        in_=cp_loc[m * 128:(m + 1) * 128,
                                   n4 * N4:(n4 + 1) * N4])
                    if first_cp_dma is None:
                        first_cp_dma = d
                        # keep the small staging inputs ahead of the 32MB
                        # class_pairs stream so prep finishes early
                        for sd in stage_dmas:
                            add_dep_helper(sd.ins, d.ins, True,
                                           "stage inputs before cp stream")
                    cpts.append(cpt)
                for k in range(K_PER_GRP):
                    ps = psv_pool.tile([128, NCH], f32, tag="psv", name="psv")
                    for m in range(M_BLK):
                        nc.tensor.matmul(ps[:], lhsT=z_loc[:, m, :],
                                         rhs=cpts[m][:, k * NCH:(k + 1) * NCH],
                                         start=(m == 0), stop=(m == M_BLK - 1))
                    col0 = n4 * N4 + k * NCH
                    xj = tmp_pool.tile([128, NCH], bf16, tag="xj", name="xj")
                    # acc_x[:, t] = sum_c ps[:, c] * zT[:, col0 + c]
                    nc.vector.scalar_tensor_tensor(
                        out=xj[:], in0=ps[:], scalar=1.0,
                        in1=zT[:, col0:col0 + NCH],
                        op0=OP.mult, op1=OP.mult,
                        accum_out=acc_x[:, n4 * K_PER_GRP + k:
                                        n4 * K_PER_GRP + k + 1])
                for m in range(M_BLK):
                    idx = n4 * M_BLK + m
                    if m < CP2_ON_DVE:
                        sj = sqj_pool.tile([128, N4], bf16, tag="sjv",
                                           name="sjv")
                        nc.vector.scalar_tensor_tensor(
                            out=sj[:], in0=cpts[m][:], scalar=1.0,
                            in1=cpts[m][:], op0=OP.mult, op1=OP.mult,
                            accum_out=acc_cp[:, idx:idx + 1])
                    else:
                        sj = sqj_pool.tile([128, N4], bf16, tag="sja",
                                           name="sja")
                        nc.scalar.activation(sj[:], cpts[m][:], AF.Square,
                                             accum_out=acc_cp[:, idx:idx + 1])

            nc.sync.dma_start(out=out_cp[:], in_=acc_cp[:])
            nc.sync.dma_start(out=out_x[:], in_=acc_x[:])

    nc.compile()
    return nc


def _get_module():
    if "nc" not in _cached:
        _cached["nc"] = _build_module()
    return _cached["nc"]


def kernel(emb_i, emb_j, class_pairs, _return_raw=False, _trace=False):
    import ml_dtypes

    emb_i = np.ascontiguousarray(emb_i, dtype=np.float32)
    emb_j = np.ascontiguousarray(emb_j, dtype=np.float32)
    class_pairs = np.ascontiguousarray(class_pairs, dtype=np.float32)
    ident = np.eye(128, dtype=ml_dtypes.bfloat16)

    def stage(a):
        # host-side shard layout: bf16 [partition, block, d] with block n
        # holding row n*128+p on partition p (see _build_module)
        n = a.shape[0] // 128
        return np.ascontiguousarray(
            a.astype(ml_dtypes.bfloat16).reshape(n, 128, D).transpose(1, 0, 2))

    emb_i_st = stage(emb_i)
    emb_j_st = stage(emb_j)

    nc = _get_module()
    in_maps = []
    for c in range(N_CORES):
        r0 = c * R_LOC
        if r0 < N:
            emb_loc = emb_i[r0:r0 + R_LOC]
        else:
            emb_loc = emb_j[r0 - N:r0 - N + R_LOC]
        in_maps.append({
            "emb_i": emb_i_st,
            "emb_j": emb_j_st,
            "emb_loc": stage(emb_loc),
            "cp_loc": np.ascontiguousarray(class_pairs[r0:r0 + R_LOC]),
            "ident": ident,
        })

    res = run_bass_kernel_spmd(nc, in_maps, list(range(N_CORES)), trace=_trace)

    G = np.zeros((128, 128), dtype=np.float64)
    sum_cp2 = 0.0
    cross = 0.0
    for c in range(N_CORES):
        G += res.results[c]["out_g"].astype(np.float64)
        sum_cp2 += res.results[c]["out_cp"].astype(np.float64).sum()
        cross += res.results[c]["out_x"].astype(np.float64).sum()
    sum_sim2 = float((G * G).sum())
    loss = (sum_sim2 - 2.0 * cross + sum_cp2) / float(TWO_N * TWO_N)
    out = np.asarray(loss, dtype=np.float32)
    if _return_raw:
        return out, res
    return out


# revision 24
# speedup vs baseline: 1.2567x; 1.0448x over previous
"""Trainium2 Bass kernel for nn_ContrastiveLossV2 (8-core SPMD).

Reference computation:
    z = l2norm(concat([emb_i, emb_j]))          # [8192, 128]
    sim = z @ z.T                               # [8192, 8192]
    loss = mean((sim - class_pairs)**2)

Instead of materializing sim and running an elementwise (sim-cp)^2 pass
(8.4M elements/core on the vector/scalar engines — slower than the DMA
roofline), expand the square:

    sum((sim-cp)^2) = sum(sim^2) - 2*sum(sim*cp) + sum(cp^2)

  * sum(sim^2)  = ||Z^T Z||_F^2   (Gram trick; per-core G_c = Z_c^T Z_c over
                  its 1024 local rows, G = sum_c G_c summed on host in f64)
  * sum(sim*cp) = sum_{d,c} V[d,c] * zT[d,c]  with  V = Z_loc^T @ CP_loc —
                  computed by the TensorEngine directly from *row-major* cp
                  tiles (contraction over the local row dim), so cp never
                  needs a transpose. The dot against zT is a fused
                  scalar_tensor_tensor (mult + accumulate) per PSUM chunk.
  * sum(cp^2)   = Square+accumulate passes, split between the scalar and
                  vector engines.

Row sharding: core c owns reps rows [c*1024, (c+1)*1024) (cores 0-3 from
emb_i, 4-7 from emb_j) and the matching 1024-row slice of class_pairs.
Everything per-core-specific arrives as data (emb_loc / cp_loc inputs), so
one SPMD program serves all 8 cores. class_pairs is cast f32->bf16 in-flight
by the SWDGE DMA; all matmuls run in bf16 with f32 PSUM accumulation. The
small replicated emb inputs are uploaded pre-staged (bf16, partition-major
device layout) so their DMA is a contiguous read. Per-core partial sums
return as small f32 tensors; the host combines them in f64. The kernel is
DMA-bound: 32MB of class_pairs per core at ~358 GB/s. The staging inputs
are explicitly ordered ahead of the class_pairs stream so the normalization
prep is off the critical path.
"""

import numpy as np

import concourse.bacc as bacc
import concourse.mybir as mybir
import concourse.tile as tile
from concourse.bass_utils import run_bass_kernel_spmd
from concourse.tile_rust import add_dep_helper

f32 = mybir.dt.float32
bf16 = mybir.dt.bfloat16
AF = mybir.ActivationFunctionType
OP = mybir.AluOpType

N_CORES = 8
N, D = 4096, 128
TWO_N = 2 * N                     # 8192
R_LOC = TWO_N // N_CORES          # 1024 rows per core
M_BLK = R_LOC // 128              # 8 local 128-row blocks
NCH = 512                         # matmul free-dim chunk (one PSUM bank)
N4 = 2048                         # cp DMA tile width (1MB f32 reads)
N_GRP = TWO_N // N4               # 4 column groups
K_PER_GRP = N4 // NCH             # 4 chunks per group
EPS = 1e-12
# how many of the 8 per-group cp^2 squares run on the vector engine
# (the rest run on the scalar engine): per-group DVE/ACT load balance knob
CP2_ON_DVE = 2

_cached = {}


def _build_module():
    nc = bacc.Bacc("TRN2", target_bir_lowering=False, debug=False,
                   num_devices=N_CORES)

    # The (small, replicated) embedding inputs are uploaded pre-staged by the
    # host: bf16, already arranged as [partition, block, d] with block n
    # holding row n*128+p on partition p — so the device DMA is a fully
    # contiguous per-partition read instead of a 512B/row gather.
    emb_i = nc.dram_tensor("emb_i", [128, N // 128, D], bf16,
                           kind="ExternalInput")
    emb_j = nc.dram_tensor("emb_j", [128, N // 128, D], bf16,
                           kind="ExternalInput")
    emb_loc = nc.dram_tensor("emb_loc", [128, M_BLK, D], bf16,
                             kind="ExternalInput")
    cp_loc = nc.dram_tensor("cp_loc", [R_LOC, TWO_N], f32, kind="ExternalInput")
    ident = nc.dram_tensor("ident", [128, 128], bf16, kind="ExternalInput")

    out_g = nc.dram_tensor("out_g", [128, 128], f32, kind="ExternalOutput")
    # groups 0..2 contribute 8 cp^2 partials each, the (half-width) last
    # group contributes 16
    out_cp = nc.dram_tensor("out_cp", [128, M_BLK * (N_GRP + 1)], f32,
                            kind="ExternalOutput")
    out_x = nc.dram_tensor("out_x", [128, N_GRP * K_PER_GRP], f32,
                           kind="ExternalOutput")

    with tile.TileContext(nc) as tc:
        with (
            tc.tile_pool(name="const", bufs=1) as const_pool,
            tc.tile_pool(name="persist", bufs=1) as persist,
            tc.tile_pool(name="stag", bufs=2) as stag_pool,
            tc.tile_pool(name="sq", bufs=1) as sq_pool,
            tc.tile_pool(name="norm", bufs=4) as norm_pool,
            tc.tile_pool(name="zc", bufs=2) as zc_pool,
            tc.tile_pool(name="cpt", bufs=22) as cp_pool,
            tc.tile_pool(name="tmp", bufs=4) as tmp_pool,
            tc.tile_pool(name="sqj", bufs=2) as sqj_pool,
            tc.tile_pool(name="psv", bufs=5, space="PSUM") as psv_pool,
            tc.tile_pool(name="pst", bufs=2, space="PSUM") as pst_pool,
            tc.tile_pool(name="psg", bufs=1, space="PSUM") as psg_pool,
        ):
            # zT[d, r] = normalized reps row r, transposed. bf16.
            zT = persist.tile([128, TWO_N], bf16)
            # local row-major z tiles (natural 128-row blocks), bf16.
            z_loc = persist.tile([128, M_BLK, 128], bf16)
            # accumulators for the partial sums
            acc_cp = persist.tile([128, M_BLK * (N_GRP + 1)], f32)
            acc_x = persist.tile([128, N_GRP * K_PER_GRP], f32)

            # All input loads go through the SWDGE (gpsimd) queue in strict
            # order: local staging first (unblocks the V matmuls), then the
            # full embs (unblock zT), then the 32MB class_pairs stream.
            stage_dmas = []
            stag_l = stag_pool.tile([128, M_BLK, 128], bf16, tag="stag_loc",
                                    name="stag_loc")
            stage_dmas.append(nc.gpsimd.dma_start(out=stag_l[:], in_=emb_loc[:]))
            stags = []
            for emb in (emb_i, emb_j):
                stag = stag_pool.tile([128, 32, 128], bf16, tag="stag",
                                      name="stag")
                stage_dmas.append(nc.gpsimd.dma_start(out=stag[:], in_=emb[:]))
                stags.append(stag)
            ident_sb = const_pool.tile([128, 128], bf16)
            stage_dmas.append(nc.gpsimd.dma_start(out=ident_sb[:], in_=ident[:]))

            def normalize(stag, n_chunks, z_out):
                """stag: [128, n_chunks, 128] bf16 staging; chunk n is the
                natural 128-row block n (row n*128+p on partition p). Writes
                the row-normalized bf16 result into z_out [128, n_chunks, 128]
                with a single broadcast multiply (f32 internal math)."""
                sq = sq_pool.tile([128, n_chunks, 128], f32, tag="sq",
                                  name="sq")
                nc.vector.tensor_tensor(sq[:], stag[:], stag[:], op=OP.mult)
                nsq = norm_pool.tile([128, n_chunks], f32, tag="nsq",
                                     name="nsq")
                nc.vector.tensor_reduce(nsq[:], sq[:], axis=mybir.AxisListType.X,
                                        op=OP.add)
                nrm = norm_pool.tile([128, n_chunks], f32, tag="nrm",
                                     name="nrm")
                nc.scalar.activation(nrm[:], nsq[:], AF.Sqrt)
                nc.vector.tensor_scalar_max(nrm[:], nrm[:], EPS)
                rec = norm_pool.tile([128, n_chunks], f32, tag="rec",
                                     name="rec")
                nc.vector.reciprocal(rec[:], nrm[:])
                rec_b = rec[:].rearrange("q n -> q n ()") \
                    .broadcast_to([128, n_chunks, 128])
                nc.vector.tensor_tensor(z_out, stag[:], rec_b, op=OP.mult)

            # ---- phase A: local row blocks (natural order) ----
            normalize(stag_l, M_BLK, z_loc[:])

            # ---- phase B: build zT from emb_i / emb_j ----
            for ei, stag in enumerate(stags):
                base = ei * N
                zbig = zc_pool.tile([128, 32, 128], bf16, tag="zbig",
                                    name="zbig")
                normalize(stag, 32, zbig[:])
                for g in range(8):
                    ps4 = pst_pool.tile([128, 4, 128], bf16, tag="ps4",
                                        name="ps4")
                    for dlt in range(4):
                        nc.tensor.transpose(ps4[:, dlt, :],
                                            zbig[:, 4 * g + dlt, :], ident_sb[:])
                    # chunks 4g..4g+3 transpose to zT columns
                    # [base+512g, base+512g+512), contiguous.
                    nc.vector.tensor_copy(
                        zT[:, base + 512 * g: base + 512 * (g + 1)]
                        .rearrange("q (n p) -> q n p", n=4),
                        ps4[:])

            # ---- phase C: G = Z_loc^T @ Z_loc (local Gram, 128x128) ----
            g_ps = psg_pool.tile([128, 128], f32)
            for m in range(M_BLK):
                nc.tensor.matmul(g_ps[:], lhsT=z_loc[:, m, :], rhs=z_loc[:, m, :],
                                 start=(m == 0), stop=(m == M_BLK - 1))
            g_sb = tmp_pool.tile([128, 128], f32, tag="gsb")
            nc.scalar.copy(g_sb[:], g_ps[:])
            nc.sync.dma_start(out=out_g[:], in_=g_sb[:])

            # ---- phase D: stream class_pairs ----
            # The last group streams in half-width tiles so the final
            # matmul/square chain after the last HBM byte is shorter.
            first_cp_dma = [None]

            def stream_group(col_base, width, acc0):
                n_k = width // NCH
                cpts = []
                for m in range(M_BLK):
                    cpt = cp_pool.tile([128, width], bf16, tag="cpt",
                                       name="cpt", padded_shape=[128, N4])
                    # SWDGE cast f32 -> bf16 in flight
                    d = nc.gpsimd.dma_start(
                        out=cpt[:],
                        in_=cp_loc[m * 128:(m + 1) * 128,
                                   col_base:col_base + width])
                    if first_cp_dma[0] is None:
                        first_cp_dma[0] = d
                        # keep the small staging inputs ahead of the 32MB
                        # class_pairs stream (pure ordering edge; same-queue
                        # FIFO then orders the transfers without a wait)
                        for sd in stage_dmas:
                            add_dep_helper(sd.ins, d.ins, False,
                                           "stage inputs before cp stream")
                    cpts.append(cpt)
                for k in range(n_k):
                    ps = psv_pool.tile([128, NCH], f32, tag="psv", name="psv")
                    for m in range(M_BLK):
                        nc.tensor.matmul(ps[:], lhsT=z_loc[:, m, :],
                                         rhs=cpts[m][:, k * NCH:(k + 1) * NCH],
                                         start=(m == 0), stop=(m == M_BLK - 1))
                    col0 = col_base + k * NCH
                    xj = tmp_pool.tile([128, NCH], bf16, tag="xj", name="xj")
                    # acc_x[:, t] = sum_c ps[:, c] * zT[:, col0 + c]
                    nc.vector.scalar_tensor_tensor(
                        out=xj[:], in0=ps[:], scalar=1.0,
                        in1=zT[:, col0:col0 + NCH],
                        op0=OP.mult, op1=OP.mult,
                        accum_out=acc_x[:, col0 // NCH:col0 // NCH + 1])
                for m in range(M_BLK):
                    idx = acc0 + m
                    if m < CP2_ON_DVE:
                        sj = sqj_pool.tile([128, width], bf16, tag="sjv",
                                           name="sjv", padded_shape=[128, N4])
                        nc.vector.scalar_tensor_tensor(
                            out=sj[:], in0=cpts[m][:], scalar=1.0,
                            in1=cpts[m][:], op0=OP.mult, op1=OP.mult,
                            accum_out=acc_cp[:, idx:idx + 1])
                    else:
                        sj = sqj_pool.tile([128, width], bf16, tag="sja",
                                           name="sja", padded_shape=[128, N4])
                        nc.scalar.activation(sj[:], cpts[m][:], AF.Square,
                                             accum_out=acc_cp[:, idx:idx + 1])

            for n4 in range(N_GRP - 1):
                stream_group(n4 * N4, N4, n4 * M_BLK)
            for h in range(2):
                stream_group((N_GRP - 1) * N4 + h * (N4 // 2), N4 // 2,
                             (N_GRP - 1) * M_BLK + h * M_BLK)

            nc.sync.dma_start(out=out_cp[:], in_=acc_cp[:])
            nc.sync.dma_start(out=out_x[:], in_=acc_x[:])

    nc.compile()
    return nc


def _get_module():
    if "nc" not in _cached:
        _cached["nc"] = _build_module()
    return _cached["nc"]


def kernel(emb_i, emb_j, class_pairs, _return_raw=False, _trace=False):
    import ml_dtypes

    emb_i = np.ascontiguousarray(emb_i, dtype=np.float32)
    emb_j = np.ascontiguousarray(emb_j, dtype=np.float32)
    class_pairs = np.ascontiguousarray(class_pairs, dtype=np.float32)
    ident = np.eye(128, dtype=ml_dtypes.bfloat16)

    def stage(a):
        # host-side shard layout: bf16 [partition, block, d] with block n
        # holding row n*128+p on partition p (see _build_module)
        n = a.shape[0] // 128
        return np.ascontiguousarray(
            a.astype(ml_dtypes.bfloat16).reshape(n, 128, D).transpose(1, 0, 2))

    emb_i_st = stage(emb_i)
    emb_j_st = stage(emb_j)

    nc = _get_module()
    in_maps = []
    for c in range(N_CORES):
        r0 = c * R_LOC
        if r0 < N:
            emb_loc = emb_i[r0:r0 + R_LOC]
        else:
            emb_loc = emb_j[r0 - N:r0 - N + R_LOC]
        in_maps.append({
            "emb_i": emb_i_st,
            "emb_j": emb_j_st,
            "emb_loc": stage(emb_loc),
            "cp_loc": np.ascontiguousarray(class_pairs[r0:r0 + R_LOC]),
            "ident": ident,
        })

    res = run_bass_kernel_spmd(nc, in_maps, list(range(N_CORES)), trace=_trace)

    G = np.zeros((128, 128), dtype=np.float64)
    sum_cp2 = 0.0
    cross = 0.0
    for c in range(N_CORES):
        G += res.results[c]["out_g"].astype(np.float64)
        sum_cp2 += res.results[c]["out_cp"].astype(np.float64).sum()
        cross += res.results[c]["out_x"].astype(np.float64).sum()
        # (out_cp has 40 columns: 8 per full group + 16 for the split last
        # group; summing all of them is exactly sum(cp^2) once per element)
    sum_sim2 = float((G * G).sum())
    loss = (sum_sim2 - 2.0 * cross + sum_cp2) / float(TWO_N * TWO_N)
    out = np.asarray(loss, dtype=np.float32)
    if _return_raw:
        return out, res
    return out
